# revision 4
# baseline (speedup 1.0000x reference)
"""Trainium2 Bass kernel for batched 3x3 VALID conv (NCHW / OIHW).

x: [32, 128, 64, 64] f32, weight: [256, 128, 3, 3] f32 -> out: [32, 256, 62, 62] f32.

Strategy: data-parallel over batch across 8 NeuronCores (4 images each).
Per core, conv is computed as 9 shift-matmuls accumulated in PSUM:
  out[co, y, x] += W[dy,dx][ci,co].T @ x[ci, y+dy, x+dx]
Operands are cast to bf16 on the host (PSUM accumulation stays fp32; the
output-norm relative error is ~3e-3). bf16 weights get the hardware fast
weight load, so the per-matmul LDWEIGHTS fully hides behind the previous
matmul's streaming (216 ns/matmul at N=496 vs 229+ ns for fp32r), and the
input-image DMA bytes halve. Every output row group uses a strided
[nr, 62] moving window, so no wasted columns are streamed and no reads go
past the image; with the first rows of image 0 landing ~1 us into the
kernel the PE starts real work immediately (no warmup matmuls needed).
"""

import numpy as np

_B, _CIN, _H, _W = 32, 128, 64, 64
_COUT = 256
_HO, _WO = 62, 62
_NCORES = 8
_BPC = _B // _NCORES  # images per core
_TAPS = 9
_GROUPS = [(r0, min(8, _HO - r0)) for r0 in range(0, _HO, 8)]

_nc_cache = None


def _build():
    global _nc_cache
    if _nc_cache is not None:
        return _nc_cache

    import concourse.bass as bass
    import concourse.mybir as mybir
    from concourse import bacc
    from concourse.tile import TileContext

    f32 = mybir.dt.float32
    bf16 = mybir.dt.bfloat16

    nc = bacc.Bacc("TRN2", target_bir_lowering=False)
    x_d = nc.dram_tensor("x", [_BPC, _CIN, _H, _W], bf16, kind="ExternalInput")
    w_d = nc.dram_tensor("w", [_CIN, _TAPS, _COUT], bf16, kind="ExternalInput")
    o_d = nc.dram_tensor("o", [_BPC, _COUT, _HO, _WO], f32, kind="ExternalOutput")

    with TileContext(nc) as tc:
        with (
            tc.tile_pool(name="wpool", bufs=1) as wpool,
            tc.tile_pool(name="xpool", bufs=2) as xpool,
            tc.tile_pool(name="spool", bufs=4) as spool,
            tc.tile_pool(name="pspool", bufs=6, space=bass.MemorySpace.PSUM) as pspool,
        ):
            w_sb = wpool.tile([_CIN, _TAPS, _COUT], bf16)
            x_tile_a = xpool.tile([_CIN, _H, _W], bf16, tag="x")
            x_tile_b = xpool.tile([_CIN, _H, _W], bf16, tag="x")
            x_tiles = [x_tile_a, x_tile_b]

            # Head DMAs, spread over three queues. The first rows of image 0
            # and the first weight-tap triple are small so the PE can start
            # group 0 as early as possible; later chunks stream in behind.
            # Image prefetches go on the gpsimd queue BEHIND image 0's row
            # chunks: queue FIFO order keeps them from stealing HBM
            # bandwidth while the head rows are still critical.
            nc.sync.dma_start(x_tiles[0][:, 0:10, :], x_d[0, :, 0:10, :])
            nc.scalar.dma_start(w_sb[:, 0:3, :], w_d[:, 0:3, :])
            nc.scalar.dma_start(w_sb[:, 3:6, :], w_d[:, 3:6, :])
            nc.scalar.dma_start(w_sb[:, 6:9, :], w_d[:, 6:9, :])
            nc.gpsimd.dma_start(x_tiles[0][:, 10:32, :], x_d[0, :, 10:32, :])
            nc.gpsimd.dma_start(x_tiles[0][:, 32:48, :], x_d[0, :, 32:48, :])
            nc.scalar.dma_start(x_tiles[0][:, 48:64, :], x_d[0, :, 48:64, :])

            def mm(ps, x_sb, ct, r0, nr, tap, start, stop):
                dy, dx = divmod(tap, 3)
                rhs = x_sb[:, r0 + dy : r0 + dy + nr, dx : dx + _WO]
                nc.tensor.matmul(
                    ps[:],
                    w_sb[:, tap, ct * 128 : (ct + 1) * 128],
                    rhs,
                    start=start,
                    stop=stop,
                )

            def finish_group(ps, img, ct, r0, nr, pipelined_tail=False):
                st = spool.tile([128, nr, _WO], f32, tag="st")
                o_slice = o_d[img, ct * 128 : (ct + 1) * 128, r0 : r0 + nr, :]
                if pipelined_tail:
                    # final groups: overlap store with copy in two halves on
                    # two independent queues to shorten the drain tail
                    h = nr // 2
                    nc.vector.tensor_copy(st[:, 0:h, :], ps[:, 0:h, :])
                    nc.sync.dma_start(o_slice[:, 0:h, :], st[:, 0:h, :])
                    nc.vector.tensor_copy(st[:, h:nr, :], ps[:, h:nr, :])
                    nc.scalar.dma_start(o_slice[:, h:nr, :], st[:, h:nr, :])
                else:
                    nc.vector.tensor_copy(st[:], ps[:])
                    nc.sync.dma_start(o_slice, st[:])

            for img in range(_BPC):
                x_sb = x_tiles[img % 2]
                last_img = img == _BPC - 1
                for ct in range(_COUT // 128):
                    # Prefetch next image while this one's second cout-tile
                    # computes (its x tile's last reader finished one full
                    # cout-tile ago, so the WAR dependency is already clear).
                    if ct == 1 and img + 1 < _BPC:
                        nxt = x_tiles[(img + 1) % 2]
                        nc.gpsimd.dma_start(nxt[:], x_d[img + 1])
                    # In the very last round drain the small 6-row group
                    # first so the final drained groups are 8-row ones.
                    groups = _GROUPS
                    if last_img and ct == 1:
                        groups = [_GROUPS[-1]] + _GROUPS[:-1]
                    for gi, (r0, nr) in enumerate(groups):
                        ps = pspool.tile([128, nr, _WO], f32, tag="ps")
                        for tap in range(_TAPS):
                            mm(ps, x_sb, ct, r0, nr, tap,
                               start=(tap == 0), stop=(tap == _TAPS - 1))
                        finish_group(
                            ps, img, ct, r0, nr,
                            pipelined_tail=(
                                last_img and ct == 1 and gi >= len(groups) - 2
                            ),
                        )

    nc.compile()
    _nc_cache = nc
    return nc


def _prep_in_maps(x, weight):
    import ml_dtypes

    bf16 = ml_dtypes.bfloat16
    x = np.asarray(x, dtype=np.float32)
    w = np.asarray(weight, dtype=np.float32)
    assert x.shape == (_B, _CIN, _H, _W), x.shape
    assert w.shape == (_COUT, _CIN, 3, 3), w.shape
    # w[ci, dy*3+dx, co] = weight[co, ci, dy, dx]
    wt = np.ascontiguousarray(
        w.transpose(1, 2, 3, 0).reshape(_CIN, _TAPS, _COUT).astype(bf16)
    )
    xs = x.reshape(_NCORES, _BPC, _CIN, _H, _W).astype(bf16)
    return [{"x": np.ascontiguousarray(xs[i]), "w": wt} for i in range(_NCORES)]


def _run(x, weight, **kwargs):
    from concourse.bass_utils import run_bass_kernel_spmd

    nc = _build()
    res = run_bass_kernel_spmd(
        nc, _prep_in_maps(x, weight), core_ids=list(range(_NCORES)), **kwargs
    )
    out = np.concatenate([r["o"] for r in res.results], axis=0)
    return out.astype(np.float32, copy=False), res


def kernel(x, weight):
    out, _ = _run(x, weight)
    return out


# revision 6
# speedup vs baseline: 1.0175x; 1.0175x over previous
"""Trainium2 Bass kernel for batched 3x3 VALID conv (NCHW / OIHW).

x: [32, 128, 64, 64] f32, weight: [256, 128, 3, 3] f32 -> out: [32, 256, 62, 62] f32.

Strategy: data-parallel over batch across 8 NeuronCores (4 images each).
Per core, conv is computed as 9 shift-matmuls accumulated in PSUM:
  out[co, y, x] += W[dy,dx][ci,co].T @ x[ci, y+dy, x+dx]
Operands are cast to bf16 on the host (PSUM accumulation stays fp32; the
output-norm relative error is ~3e-3). bf16 weights get the hardware fast
weight load, so the per-matmul LDWEIGHTS fully hides behind the previous
matmul's streaming (216 ns/matmul at N=496 vs 229+ ns for fp32r), and the
input-image DMA bytes halve. Every output row group uses a strided
[nr, 62] moving window, so no wasted columns are streamed and no reads go
past the image; with the first rows of image 0 landing ~1 us into the
kernel the PE starts real work immediately (no warmup matmuls needed).
"""

import numpy as np

_B, _CIN, _H, _W = 32, 128, 64, 64
_COUT = 256
_HO, _WO = 62, 62
_NCORES = 8
_BPC = _B // _NCORES  # images per core
_TAPS = 9
_GROUPS = [(r0, min(8, _HO - r0)) for r0 in range(0, _HO, 8)]

_nc_cache = None


def _build():
    global _nc_cache
    if _nc_cache is not None:
        return _nc_cache

    import concourse.bass as bass
    import concourse.mybir as mybir
    from concourse import bacc
    from concourse.tile import TileContext

    f32 = mybir.dt.float32
    bf16 = mybir.dt.bfloat16

    nc = bacc.Bacc("TRN2", target_bir_lowering=False)
    x_d = nc.dram_tensor("x", [_BPC, _CIN, _H, _W], bf16, kind="ExternalInput")
    w_d = nc.dram_tensor("w", [_CIN, _TAPS, _COUT], bf16, kind="ExternalInput")
    o_d = nc.dram_tensor("o", [_BPC, _COUT, _HO, _WO], f32, kind="ExternalOutput")

    with TileContext(nc) as tc:
        with (
            tc.tile_pool(name="wpool", bufs=1) as wpool,
            tc.tile_pool(name="xpool", bufs=2) as xpool,
            tc.tile_pool(name="spool", bufs=4) as spool,
            tc.tile_pool(name="pspool", bufs=6, space=bass.MemorySpace.PSUM) as pspool,
        ):
            w_sb = wpool.tile([_CIN, _TAPS, _COUT], bf16)
            x_tile_a = xpool.tile([_CIN, _H, _W], bf16, tag="x")
            x_tile_b = xpool.tile([_CIN, _H, _W], bf16, tag="x")
            x_tiles = [x_tile_a, x_tile_b]

            # Head DMAs, spread over three queues (per-queue DMA throughput
            # is only ~150 GB/s, so critical head bytes must be spread).
            # Weights ride the sync queue, which spins up first; it is free
            # again well before the first output store needs it. Image
            # prefetches go on the gpsimd queue BEHIND all of image 0's row
            # chunks: queue FIFO order keeps them from stealing HBM
            # bandwidth while the head bytes are still critical.
            nc.sync.dma_start(w_sb[:, 0:3, :], w_d[:, 0:3, :])
            nc.sync.dma_start(w_sb[:, 3:6, :], w_d[:, 3:6, :])
            nc.sync.dma_start(w_sb[:, 6:9, :], w_d[:, 6:9, :])
            nc.scalar.dma_start(x_tiles[0][:, 0:10, :], x_d[0, :, 0:10, :])
            nc.gpsimd.dma_start(x_tiles[0][:, 10:32, :], x_d[0, :, 10:32, :])
            nc.gpsimd.dma_start(x_tiles[0][:, 32:48, :], x_d[0, :, 32:48, :])
            nc.gpsimd.dma_start(x_tiles[0][:, 48:64, :], x_d[0, :, 48:64, :])

            def mm(ps, x_sb, ct, r0, nr, tap, start, stop):
                dy, dx = divmod(tap, 3)
                rhs = x_sb[:, r0 + dy : r0 + dy + nr, dx : dx + _WO]
                nc.tensor.matmul(
                    ps[:],
                    w_sb[:, tap, ct * 128 : (ct + 1) * 128],
                    rhs,
                    start=start,
                    stop=stop,
                )

            def finish_group(ps, img, ct, r0, nr, pipelined_tail=False):
                st = spool.tile([128, nr, _WO], f32, tag="st")
                o_slice = o_d[img, ct * 128 : (ct + 1) * 128, r0 : r0 + nr, :]
                if pipelined_tail:
                    # final groups: overlap store with copy in two pieces
                    # (large first, small last) on the warm sync queue to
                    # shorten the drain tail. A second queue does not help
                    # here: an idle DMA queue takes ~1.5us to spin back up.
                    h = nr - 2
                    nc.vector.tensor_copy(st[:, 0:h, :], ps[:, 0:h, :])
                    nc.sync.dma_start(o_slice[:, 0:h, :], st[:, 0:h, :])
                    nc.vector.tensor_copy(st[:, h:nr, :], ps[:, h:nr, :])
                    nc.sync.dma_start(o_slice[:, h:nr, :], st[:, h:nr, :])
                else:
                    nc.vector.tensor_copy(st[:], ps[:])
                    nc.sync.dma_start(o_slice, st[:])

            for img in range(_BPC):
                x_sb = x_tiles[img % 2]
                last_img = img == _BPC - 1
                for ct in range(_COUT // 128):
                    # Prefetch next image while this one's second cout-tile
                    # computes (its x tile's last reader finished one full
                    # cout-tile ago, so the WAR dependency is already clear).
                    if ct == 1 and img + 1 < _BPC:
                        nxt = x_tiles[(img + 1) % 2]
                        nc.gpsimd.dma_start(nxt[:], x_d[img + 1])
                    # In the very last round drain the small 6-row group
                    # first so the final drained groups are 8-row ones.
                    groups = _GROUPS
                    if last_img and ct == 1:
                        groups = [_GROUPS[-1]] + _GROUPS[:-1]
                    for gi, (r0, nr) in enumerate(groups):
                        ps = pspool.tile([128, nr, _WO], f32, tag="ps")
                        for tap in range(_TAPS):
                            mm(ps, x_sb, ct, r0, nr, tap,
                               start=(tap == 0), stop=(tap == _TAPS - 1))
                        finish_group(
                            ps, img, ct, r0, nr,
                            pipelined_tail=(
                                last_img and ct == 1 and gi >= len(groups) - 2
                            ),
                        )

    nc.compile()
    _nc_cache = nc
    return nc


def _prep_in_maps(x, weight):
    import ml_dtypes

    bf16 = ml_dtypes.bfloat16
    x = np.asarray(x, dtype=np.float32)
    w = np.asarray(weight, dtype=np.float32)
    assert x.shape == (_B, _CIN, _H, _W), x.shape
    assert w.shape == (_COUT, _CIN, 3, 3), w.shape
    # w[ci, dy*3+dx, co] = weight[co, ci, dy, dx]
    wt = np.ascontiguousarray(
        w.transpose(1, 2, 3, 0).reshape(_CIN, _TAPS, _COUT).astype(bf16)
    )
    xs = x.reshape(_NCORES, _BPC, _CIN, _H, _W).astype(bf16)
    return [{"x": np.ascontiguousarray(xs[i]), "w": wt} for i in range(_NCORES)]


def _run(x, weight, **kwargs):
    from concourse.bass_utils import run_bass_kernel_spmd

    nc = _build()
    res = run_bass_kernel_spmd(
        nc, _prep_in_maps(x, weight), core_ids=list(range(_NCORES)), **kwargs
    )
    out = np.concatenate([r["o"] for r in res.results], axis=0)
    return out.astype(np.float32, copy=False), res


def kernel(x, weight):
    out, _ = _run(x, weight)
    return out


# revision 9
# speedup vs baseline: 1.0396x; 1.0216x over previous
"""Trainium2 Bass kernel for batched 3x3 VALID conv (NCHW / OIHW).

x: [32, 128, 64, 64] f32, weight: [256, 128, 3, 3] f32 -> out: [32, 256, 62, 62] f32.

Strategy: data-parallel over batch across 8 NeuronCores (4 images each).
Per core, conv is computed as 9 shift-matmuls accumulated in PSUM:
  out[co, y, x] += W[dy,dx][ci,co].T @ x[ci, y+dy, x+dx]
Operands are cast to bf16 on the host (PSUM accumulation stays fp32; the
output-norm relative error is ~3e-3). bf16 weights get the hardware fast
weight load, so the per-matmul LDWEIGHTS fully hides behind the previous
matmul's streaming (216 ns/matmul at N=496 vs 229+ ns for fp32r), and the
input-image DMA bytes halve. Every output row group uses a strided
[nr, 62] moving window, so no wasted columns are streamed and no reads go
past the image; with the first rows of image 0 landing ~1 us into the
kernel the PE starts real work immediately (no warmup matmuls needed).
"""

import numpy as np

_B, _CIN, _H, _W = 32, 128, 64, 64
_COUT = 256
_HO, _WO = 62, 62
_NCORES = 8
_BPC = _B // _NCORES  # images per core
_TAPS = 9
_GROUPS = [(r0, min(8, _HO - r0)) for r0 in range(0, _HO, 8)]

_nc_cache = None


def _build():
    global _nc_cache
    if _nc_cache is not None:
        return _nc_cache

    import concourse.bass as bass
    import concourse.mybir as mybir
    from concourse import bacc
    from concourse.tile import TileContext

    f32 = mybir.dt.float32
    bf16 = mybir.dt.bfloat16

    nc = bacc.Bacc("TRN2", target_bir_lowering=False)
    x_d = nc.dram_tensor("x", [_BPC, _CIN, _H, _W], bf16, kind="ExternalInput")
    w_d = nc.dram_tensor("w", [_CIN, _TAPS, _COUT], bf16, kind="ExternalInput")
    o_d = nc.dram_tensor("o", [_BPC, _COUT, _HO, _WO], f32, kind="ExternalOutput")

    with TileContext(nc) as tc:
        with (
            tc.tile_pool(name="wpool", bufs=1) as wpool,
            tc.tile_pool(name="xpool", bufs=2) as xpool,
            tc.tile_pool(name="spool", bufs=4) as spool,
            tc.tile_pool(name="pspool", bufs=6, space=bass.MemorySpace.PSUM) as pspool,
        ):
            w_sb = wpool.tile([_CIN, _TAPS, _COUT], bf16)
            x_tile_a = xpool.tile([_CIN, _H, _W], bf16, tag="x")
            x_tile_b = xpool.tile([_CIN, _H, _W], bf16, tag="x")
            x_tiles = [x_tile_a, x_tile_b]

            # Head DMAs, spread over three queues (per-queue DMA throughput
            # is only ~150 GB/s, so critical head bytes must be spread).
            # Weights ride the sync queue, which spins up first; it is free
            # again well before the first output store needs it. Image
            # prefetches go on the gpsimd queue BEHIND all of image 0's row
            # chunks: queue FIFO order keeps them from stealing HBM
            # bandwidth while the head bytes are still critical.
            nc.sync.dma_start(w_sb[:, 0:3, :], w_d[:, 0:3, :])
            nc.sync.dma_start(w_sb[:, 3:6, :], w_d[:, 3:6, :])
            nc.sync.dma_start(w_sb[:, 6:9, :], w_d[:, 6:9, :])
            nc.scalar.dma_start(x_tiles[0][:, 0:10, :], x_d[0, :, 0:10, :])
            nc.scalar.dma_start(x_tiles[0][:, 10:18, :], x_d[0, :, 10:18, :])
            nc.gpsimd.dma_start(x_tiles[0][:, 18:34, :], x_d[0, :, 18:34, :])
            nc.gpsimd.dma_start(x_tiles[0][:, 34:64, :], x_d[0, :, 34:64, :])

            def mm(ps, x_sb, ct, r0, nr, tap, start, stop):
                dy, dx = divmod(tap, 3)
                rhs = x_sb[:, r0 + dy : r0 + dy + nr, dx : dx + _WO]
                nc.tensor.matmul(
                    ps[:],
                    w_sb[:, tap, ct * 128 : (ct + 1) * 128],
                    rhs,
                    start=start,
                    stop=stop,
                )

            def finish_group(ps, img, ct, r0, nr, pipelined_tail=False):
                st = spool.tile([128, nr, _WO], f32, tag="st")
                o_slice = o_d[img, ct * 128 : (ct + 1) * 128, r0 : r0 + nr, :]
                if pipelined_tail:
                    # final groups: overlap store with copy in two pieces
                    # (large first, small last) on the warm sync queue to
                    # shorten the drain tail. A second queue does not help
                    # here: an idle DMA queue takes ~1.5us to spin back up.
                    h = nr - 2
                    nc.vector.tensor_copy(st[:, 0:h, :], ps[:, 0:h, :])
                    nc.sync.dma_start(o_slice[:, 0:h, :], st[:, 0:h, :])
                    nc.vector.tensor_copy(st[:, h:nr, :], ps[:, h:nr, :])
                    nc.sync.dma_start(o_slice[:, h:nr, :], st[:, h:nr, :])
                else:
                    nc.vector.tensor_copy(st[:], ps[:])
                    nc.sync.dma_start(o_slice, st[:])

            for img in range(_BPC):
                x_sb = x_tiles[img % 2]
                last_img = img == _BPC - 1
                for ct in range(_COUT // 128):
                    # Prefetch next image while this one's second cout-tile
                    # computes (its x tile's last reader finished one full
                    # cout-tile ago, so the WAR dependency is already clear).
                    if ct == 1 and img + 1 < _BPC:
                        nxt = x_tiles[(img + 1) % 2]
                        nc.gpsimd.dma_start(nxt[:], x_d[img + 1])
                    if img == 0 and ct == 0:
                        # First round: interleave the first three groups by
                        # weight-tap triple so the PE consumes w chunks in
                        # DMA arrival order with 3x-relaxed deadlines (the
                        # serial w chain only gets a ~100-150 GB/s queue
                        # share while image-0 row chunks stream in parallel).
                        head_ps = [
                            pspool.tile([128, 8, _WO], f32, tag="ps",
                                        name=f"head_ps{i}")
                            for i in range(3)
                        ]
                        for t0 in range(0, _TAPS, 3):
                            for gi3, ps in enumerate(head_ps):
                                for tap in range(t0, t0 + 3):
                                    mm(ps, x_sb, 0, gi3 * 8, 8, tap,
                                       start=(tap == 0), stop=(tap == _TAPS - 1))
                        for gi3, ps in enumerate(head_ps):
                            finish_group(ps, 0, 0, gi3 * 8, 8)
                        groups = _GROUPS[3:]
                    elif last_img and ct == 1:
                        # In the very last round drain the small 6-row group
                        # first so the final drained groups are 8-row ones.
                        groups = [_GROUPS[-1]] + _GROUPS[:-1]
                    else:
                        groups = _GROUPS
                    for gi, (r0, nr) in enumerate(groups):
                        ps = pspool.tile([128, nr, _WO], f32, tag="ps")
                        for tap in range(_TAPS):
                            mm(ps, x_sb, ct, r0, nr, tap,
                               start=(tap == 0), stop=(tap == _TAPS - 1))
                        finish_group(
                            ps, img, ct, r0, nr,
                            pipelined_tail=(
                                last_img and ct == 1 and gi >= len(groups) - 2
                            ),
                        )

    nc.compile()
    _nc_cache = nc
    return nc


def _prep_in_maps(x, weight):
    import ml_dtypes

    bf16 = ml_dtypes.bfloat16
    x = np.asarray(x, dtype=np.float32)
    w = np.asarray(weight, dtype=np.float32)
    assert x.shape == (_B, _CIN, _H, _W), x.shape
    assert w.shape == (_COUT, _CIN, 3, 3), w.shape
    # w[ci, dy*3+dx, co] = weight[co, ci, dy, dx]
    wt = np.ascontiguousarray(
        w.transpose(1, 2, 3, 0).reshape(_CIN, _TAPS, _COUT).astype(bf16)
    )
    xs = x.reshape(_NCORES, _BPC, _CIN, _H, _W).astype(bf16)
    return [{"x": np.ascontiguousarray(xs[i]), "w": wt} for i in range(_NCORES)]


def _run(x, weight, **kwargs):
    from concourse.bass_utils import run_bass_kernel_spmd

    nc = _build()
    res = run_bass_kernel_spmd(
        nc, _prep_in_maps(x, weight), core_ids=list(range(_NCORES)), **kwargs
    )
    out = np.concatenate([r["o"] for r in res.results], axis=0)
    return out.astype(np.float32, copy=False), res


def kernel(x, weight):
    out, _ = _run(x, weight)
    return out
